# revision 1
# baseline (speedup 1.0000x reference)
"""Trainium2 Bass kernel for nn_CrossEmbed2GraphByProduct (segment_reduce).

For embeddings [B=512, R=264, K=32] and segment ends [S=9] computes
  adjacency[b] = E_b @ E_b^T                                   [B, R, R, 1]
  intra        = adjacency * blockdiag_mask                    [B, R, R, 1]
  inter[b]     = seg-block sums of adjacency[b] / counts       [B, S, S, 1]
returned as (intra, inter, adjacency) like the reference.

Strategy: pure data parallel over 8 NeuronCores (64 batches each).
Per batch the [264, 264] gram matrix is built with 3 row-tile matmuls
(contraction K=32 on the partition dim).  ScalarE copies adjacency
PSUM->SBUF, VectorE applies the block-diagonal mask for intra, and
inter = P @ P^T / counts where P[s] = sum of embedding rows in segment s
(vector segment reduces batched across a group).  Outputs are DMA'd in
~1MB chunks (8 batches per transfer) to stay near HBM line rate.
"""

import sys

import numpy as np

for _p in ("/opt/trn_rl_repo",):
    if _p not in sys.path:
        sys.path.append(_p)

B, R, K = 512, 264, 32
N_CORES = 8
BPC = B // N_CORES  # 64 batches per core
RT = (R + 127) // 128  # 3 row tiles: 128, 128, 8

_cache: dict = {}


def _build_nc(ends: tuple, bpc: int, nb: int):
    """Trace + compile the per-core Bass program.

    ends: segment end offsets (trace-time constants; only the reduce slice
    bounds depend on them).  bpc: batches this core processes.  nb: batches
    per DMA group.
    """
    import concourse.mybir as mybir
    import concourse.tile as tile
    from concourse import bacc

    f32 = mybir.dt.float32
    S = len(ends)
    starts = (0,) + ends[:-1]
    sizes = tuple(e - s for s, e in zip(starts, ends))
    ng = bpc // nb
    assert ng * nb == bpc

    nc = bacc.Bacc("TRN2", target_bir_lowering=False, debug=False)

    embt = nc.dram_tensor("embt", [K, bpc * R], f32, kind="ExternalInput")
    maskt = nc.dram_tensor("maskt", [128, RT * R], f32, kind="ExternalInput")
    recip = nc.dram_tensor("recip", [S, S], f32, kind="ExternalInput")
    adj = nc.dram_tensor("adj", [bpc, R, R], f32, kind="ExternalOutput")
    intra = nc.dram_tensor("intra", [bpc, R, R], f32, kind="ExternalOutput")
    inter = nc.dram_tensor("inter", [bpc, S, S], f32, kind="ExternalOutput")

    with tile.TileContext(nc) as tc:
        with (
            tc.tile_pool(name="consts", bufs=1) as consts,
            tc.tile_pool(name="emb", bufs=2) as emb_pool,
            tc.tile_pool(name="a_out", bufs=2) as a_pool,
            tc.tile_pool(name="i_out", bufs=2) as i_pool,
            tc.tile_pool(name="psums", bufs=2) as p_pool,
            tc.tile_pool(name="inter_acc", bufs=1) as inter_pool,
            tc.tile_pool(name="psum", bufs=6, space="PSUM") as psum_pool,
            tc.tile_pool(name="psum_i", bufs=2, space="PSUM") as psum_i_pool,
        ):
            mask_sb = consts.tile([128, RT * R], f32)
            nc.sync.dma_start(mask_sb[:, :], maskt[:, :])
            recip_sb = consts.tile([S, S], f32)
            nc.sync.dma_start(recip_sb[:, :], recip[:, :])
            inter_sb = inter_pool.tile([S, bpc * S], f32)

            m3 = mask_sb[:, :].rearrange("p (t c) -> p t c", c=R)

            for g in range(ng):
                e_sb = emb_pool.tile([K, nb * R], f32)
                nc.sync.dma_start(e_sb[:, :], embt[:, g * nb * R : (g + 1) * nb * R])
                a_sb = a_pool.tile([128, RT * nb * R], f32)
                i_sb = i_pool.tile([128, RT * nb * R], f32)
                p_sb = p_pool.tile([K, nb * S], f32)

                e3 = e_sb[:, :].rearrange("k (b c) -> k b c", c=R)
                p3 = p_sb[:, :].rearrange("k (b s) -> k b s", s=S)
                # segment sums for all nb batches at once, one reduce per segment
                for s in range(S):
                    if sizes[s] == 0:
                        nc.vector.memset(p3[:, :, s], 0.0)
                    else:
                        nc.vector.reduce_sum(
                            p3[:, :, s],
                            e3[:, :, starts[s] : ends[s]],
                            axis=mybir.AxisListType.X,
                        )

                a4 = a_sb[:, :].rearrange("p (t b c) -> p t b c", t=RT, b=nb)
                i4 = i_sb[:, :].rearrange("p (t b c) -> p t b c", t=RT, b=nb)

                for bb in range(nb):
                    base = bb * R
                    pts = []
                    for t in range(RT):
                        rows = min(128, R - t * 128)
                        pt = psum_pool.tile([128, R], f32)
                        nc.tensor.matmul(
                            pt[:rows, :],
                            e_sb[:, base + t * 128 : base + t * 128 + rows],
                            e_sb[:, base : base + R],
                            start=True,
                            stop=True,
                        )
                        pts.append((pt, rows))
                    for t, (pt, rows) in enumerate(pts):
                        nc.scalar.copy(a4[:rows, t, bb, :], pt[:rows, :])
                        nc.vector.tensor_mul(
                            i4[:rows, t, bb, :], pt[:rows, :], m3[:rows, t, :]
                        )
                    ip = psum_i_pool.tile([S, S], f32)
                    nc.tensor.matmul(
                        ip[:, :],
                        p_sb[:, bb * S : (bb + 1) * S],
                        p_sb[:, bb * S : (bb + 1) * S],
                        start=True,
                        stop=True,
                    )
                    gb = g * nb + bb
                    nc.vector.tensor_mul(
                        inter_sb[:, gb * S : (gb + 1) * S], ip[:, :], recip_sb[:, :]
                    )

                for t in range(RT):
                    rows = min(128, R - t * 128)
                    dram = adj[g * nb : (g + 1) * nb, t * 128 : t * 128 + rows, :]
                    nc.sync.dma_start(
                        dram.rearrange("b p c -> p b c"), a4[:rows, t]
                    )
                    dram = intra[g * nb : (g + 1) * nb, t * 128 : t * 128 + rows, :]
                    nc.sync.dma_start(
                        dram.rearrange("b p c -> p b c"), i4[:rows, t]
                    )

            nc.sync.dma_start(
                inter[:, :, :].rearrange("b s t -> s b t"),
                inter_sb[:, :].rearrange("s (b t) -> s b t", t=S),
            )

    nc.compile()
    return nc


def _host_consts(ends: tuple):
    S = len(ends)
    starts = (0,) + ends[:-1]
    sizes = np.array([e - s for s, e in zip(starts, ends)], dtype=np.int64)
    seg = np.repeat(np.arange(S), sizes)
    assert seg.shape[0] == R, f"segment ends must cover R={R}, got {ends}"
    mask = (seg[:, None] == seg[None, :]).astype(np.float32)
    maskt = np.zeros((128, RT * R), dtype=np.float32)
    for t in range(RT):
        rows = min(128, R - t * 128)
        maskt[:rows, t * R : t * R + R] = mask[t * 128 : t * 128 + rows]
    counts = (sizes[:, None] * sizes[None, :]).astype(np.float32)
    with np.errstate(divide="ignore"):
        recip = (1.0 / counts).astype(np.float32)
    return maskt, recip


def _run(embeddings, subnetwork_ends, trace=False):
    from concourse.bass_utils import run_bass_kernel_spmd

    emb = np.ascontiguousarray(np.asarray(embeddings, dtype=np.float32))
    assert emb.shape == (B, R, K), emb.shape
    ends = tuple(int(x) for x in np.asarray(subnetwork_ends).reshape(-1))

    key = ends
    if key not in _cache:
        _cache[key] = _build_nc(ends, BPC, 8)
    nc = _cache[key]

    maskt, recip = _host_consts(ends)
    in_maps = []
    for c in range(N_CORES):
        ec = emb[c * BPC : (c + 1) * BPC]  # [BPC, R, K]
        embt_c = np.ascontiguousarray(ec.transpose(2, 0, 1)).reshape(K, BPC * R)
        in_maps.append({"embt": embt_c, "maskt": maskt, "recip": recip})

    res = run_bass_kernel_spmd(
        nc, in_maps, core_ids=list(range(N_CORES)), trace=trace
    )
    adjacency = np.concatenate([r["adj"] for r in res.results], axis=0)
    intra = np.concatenate([r["intra"] for r in res.results], axis=0)
    inter = np.concatenate([r["inter"] for r in res.results], axis=0)
    return (intra[..., None], inter[..., None], adjacency[..., None]), res


def kernel(embeddings, subnetwork_ends):
    outs, _ = _run(embeddings, subnetwork_ends)
    return outs
